# revision 3
# baseline (speedup 1.0000x reference)
"""GQA attention Trainium2 kernel v2 (8 NeuronCores, SPMD, no collectives).

Sharding: 2-way data parallel (batch) x 4-way tensor parallel (heads).
Core c handles batch b=c//4 and head-group g=c%4 (8 q heads, 2 kv heads).

v2 changes vs baseline:
- reciprocal_approx_fast instead of iterative reciprocal (was 171us busy).
- Attention restructured q-block-major with head pairs (kv0+kv1) so score
  matmuls pack into disjoint PE row groups and run concurrently.
- exp calls merged across the head pair; diagonal blocks trimmed so
  fully-masked columns are neither exp'd nor streamed through PV.
- Causal mask via precomputed [128,128] triangular mask + vector multiply
  (replaces per-block gpsimd affine_select).
- Gate projection and 3/4 of o_proj interleaved into the attention phase
  as tensor-engine filler (attention alone is scalar-bound).
- sigmoid(x) computed as 0.5*tanh(x/2)+0.5 to stay in the exp activation
  table set (avoids ACT_TABLE_LOAD thrash).
- Phase-1 elementwise work in bf16 (2x DVE mode); bf16 output.
"""

import os
import sys
import numpy as np

for _p in ("/opt/trn_rl_repo", "/root/.axon_site/_ro/trn_rl_repo"):
    if os.path.isdir(_p) and _p not in sys.path:
        sys.path.insert(0, _p)

import ml_dtypes

B, S, HID = 2, 2048, 2048
NH, NKV, HD = 32, 8, 64
ROPE = 32
SCALE = HD ** -0.5
NCORES = 8
QH = NH // 4      # 8 q heads per core
KVH = NKV // 4    # 2 kv heads per core
QD = QH * HD      # 512 per-core q dim
KD = KVH * HD     # 128 per-core kv dim
KC = HID // 128   # 16 contraction chunks
SB = S // 512     # 4 sequence blocks of 512
BF16 = ml_dtypes.bfloat16

_CACHE = {}


def _build_bass(debug_dump=False):
    import concourse.bass as bass
    from concourse import bacc, mybir, tile
    from concourse.masks import make_identity

    f32 = mybir.dt.float32
    bf16 = mybir.dt.bfloat16

    nc = bacc.Bacc("TRN2", target_bir_lowering=False, debug=False,
                   enable_asserts=False, num_devices=NCORES)

    hT = nc.dram_tensor("hT", [HID, S], bf16, kind="ExternalInput").ap()
    wqT = nc.dram_tensor("wqT", [HID, QD], bf16, kind="ExternalInput").ap()
    wkT = nc.dram_tensor("wkT", [HID, KD], bf16, kind="ExternalInput").ap()
    wvT = nc.dram_tensor("wvT", [HID, KD], bf16, kind="ExternalInput").ap()
    wgT = nc.dram_tensor("wgT", [HID, QD], bf16, kind="ExternalInput").ap()
    woT = nc.dram_tensor("woT", [QD, HID], bf16, kind="ExternalInput").ap()
    csAq = nc.dram_tensor("csAq", [128, S], bf16, kind="ExternalInput").ap()
    csBq = nc.dram_tensor("csBq", [128, S], bf16, kind="ExternalInput").ap()
    csAk = nc.dram_tensor("csAk", [128, S], bf16, kind="ExternalInput").ap()
    csBk = nc.dram_tensor("csBk", [128, S], bf16, kind="ExternalInput").ap()
    outT = nc.dram_tensor("outT", [HID, S], bf16, kind="ExternalOutput").ap()
    if debug_dump:
        dbg_q = nc.dram_tensor("dbg_q", [128, 4, S], bf16, kind="ExternalOutput").ap()
        dbg_k = nc.dram_tensor("dbg_k", [128, S], bf16, kind="ExternalOutput").ap()
        dbg_v = nc.dram_tensor("dbg_v", [128, KC, KVH, HD + 1], bf16,
                               kind="ExternalOutput").ap()
        dbg_g = nc.dram_tensor("dbg_g", [128, 4, S], bf16, kind="ExternalOutput").ap()
        dbg_og = nc.dram_tensor("dbg_og", [128, 4, S], bf16,
                                kind="ExternalOutput").ap()
        dbg_wsc = nc.dram_tensor("dbg_wsc", [128, 2, 512], bf16,
                                 kind="ExternalOutput").ap()

    Exp = mybir.ActivationFunctionType.Exp
    Tanh = mybir.ActivationFunctionType.Tanh
    Square = mybir.ActivationFunctionType.Square
    Sqrt = mybir.ActivationFunctionType.Sqrt
    PSUM = bass.MemorySpace.PSUM

    with tile.TileContext(nc) as tc:
        with tc.tile_pool(name="persist", bufs=1) as pp:
            # head h: partition rows (h//4)*64, free chunk h%4 (q side);
            # og/gate: head h at rows (h%2)*64, chunk h//2
            qT_sb = pp.tile([128, 4, S], bf16)
            kT_sb = pp.tile([128, S], bf16)
            g_sb = pp.tile([128, 4, S], bf16)
            v_sb = pp.tile([128, KC, KVH, HD + 1], bf16)
            wo_sb = pp.tile([128, 4, KC, 128], bf16)
            og_sb = pp.tile([128, 4, S], bf16)
            tri = pp.tile([128, 128], bf16)          # tri[r,c]=1 if c>=r

            nc.vector.memset(v_sb[:, :, :, HD:HD + 1], 1.0)
            # tri: +inf on keep (c>=r), 0 on mask; applied via tensor-min so
            # masked inf/garbage becomes 0 instead of NaN (inf*0)
            nc.vector.memset(tri, float("inf"))
            nc.gpsimd.affine_select(out=tri, in_=tri,
                                    compare_op=mybir.AluOpType.is_ge,
                                    fill=0.0, base=0, channel_multiplier=-1,
                                    pattern=[[1, 128]])

            # ================= phase 1: q/k/v projections =================
            with tc.tile_pool(name="consts", bufs=1) as cp, \
                 tc.tile_pool(name="wts", bufs=1) as wp, \
                 tc.tile_pool(name="hblk", bufs=2) as hp_pool, \
                 tc.tile_pool(name="work", bufs=3) as wk, \
                 tc.tile_pool(name="smallw", bufs=3) as smp, \
                 tc.tile_pool(name="rbpool", bufs=2) as rbp, \
                 tc.tile_pool(name="rwork", bufs=3) as rwk, \
                 tc.tile_pool(name="pps", bufs=2, space=PSUM) as pps, \
                 tc.tile_pool(name="sqps", bufs=2, space=PSUM) as sqps, \
                 tc.tile_pool(name="bcps", bufs=2, space=PSUM) as bcps, \
                 tc.tile_pool(name="trps", bufs=2, space=PSUM) as trps:

                csA_q = cp.tile([128, S], bf16)
                csB_q = cp.tile([128, S], bf16)
                csA_k = cp.tile([128, S], bf16)
                csB_k = cp.tile([128, S], bf16)
                nc.sync.dma_start(out=csA_q, in_=csAq)
                nc.sync.dma_start(out=csB_q, in_=csBq)
                nc.sync.dma_start(out=csA_k, in_=csAk)
                nc.sync.dma_start(out=csB_k, in_=csBk)
                ident = cp.tile([128, 128], bf16)
                make_identity(nc, ident)
                ones2 = cp.tile([128, 2], bf16)
                nc.vector.memset(ones2, 0.0)
                nc.vector.memset(ones2[0:64, 0:1], 1.0)
                nc.vector.memset(ones2[64:128, 1:2], 1.0)
                # bsel: matmul-broadcast selector; out row m <- rstd row (m>=64)
                bsel = cp.tile([2, 128], bf16)
                nc.vector.memset(bsel, 0.0)
                nc.vector.memset(bsel[0:1, 0:64], 1.0)
                # partition base 1 is not DVE-addressable; fill row 1 via DMA
                nc.gpsimd.dma_start(out=bsel[1:2, 64:128], in_=bsel[0:1, 0:64])

                wq_sb = wp.tile([128, KC, QD], bf16)
                wk_sb = wp.tile([128, KC, KD], bf16)
                wv_sb = wp.tile([128, KC, KD], bf16)
                nc.sync.dma_start(out=wq_sb,
                                  in_=wqT.rearrange("(c p) m -> p c m", p=128))
                nc.sync.dma_start(out=wk_sb,
                                  in_=wkT.rearrange("(c p) m -> p c m", p=128))
                nc.sync.dma_start(out=wv_sb,
                                  in_=wvT.rearrange("(c p) m -> p c m", p=128))
                nc.sync.dma_start(out=wo_sb,
                                  in_=woT.rearrange("(c p) (mb mm) -> p c mb mm",
                                                    p=128, mm=128))

                def rope_norm(ps, csA, csB, sq_ps, rb_ps, dst0, dst1):
                    """ps: psum [128,512] raw proj (2 heads of 64 dims).
                    Writes normalized+roped bf16 rows to dst0 (rows 0:64)
                    and dst1 (rows 64:128)."""
                    # sum of squares per head-half via ones2 matmul on bf16
                    # square tile
                    sq_t = wk.tile([128, 512], bf16, tag="sq")
                    nc.scalar.activation(out=sq_t, in_=ps, func=Square)
                    qsb = rwk.tile([128, 512], bf16, tag="qsb")
                    nc.scalar.copy(out=qsb, in_=ps)
                    nc.tensor.matmul(sq_ps, ones2, sq_t, start=True, stop=True)
                    # rstd = sqrt(HD / sum_sq)
                    inv_ms = smp.tile([2, 512], f32, tag="invms")
                    nc.vector.reciprocal_approx_fast(out=inv_ms, in_=sq_ps)
                    rstdb = smp.tile([2, 512], bf16, tag="rstdb")
                    nc.scalar.activation(out=rstdb, in_=inv_ms, func=Sqrt,
                                         scale=float(HD))
                    # broadcast rstd rows to 64-row halves via tiny matmul
                    nc.tensor.matmul(rb_ps, bsel, rstdb, start=True, stop=True)
                    rb = rbp.tile([128, 512], bf16, tag="rb")
                    nc.scalar.copy(out=rb, in_=rb_ps)
                    # rope: qa = qsb*csA + rot(qsb)*csB  (bf16, 2x DVE)
                    qa = rwk.tile([128, 512], bf16, tag="qa")
                    nc.vector.tensor_mul(qa, qsb, csA)
                    rot = rwk.tile([128, 512], bf16, tag="rot")
                    nc.vector.memset(rot[32:64, :], 0.0)
                    nc.vector.memset(rot[96:128, :], 0.0)
                    for hh in (0, 64):
                        nc.gpsimd.dma_start(out=rot[hh + 0:hh + 16],
                                            in_=qsb[hh + 16:hh + 32])
                        nc.gpsimd.dma_start(out=rot[hh + 16:hh + 32],
                                            in_=qsb[hh + 0:hh + 16])
                    nc.vector.tensor_mul(rot, rot, csB)
                    nc.vector.tensor_add(qa, qa, rot)
                    nc.vector.tensor_mul(dst0, qa[0:64, :], rb[0:64, :])
                    nc.vector.tensor_mul(dst1, qa[64:128, :], rb[64:128, :])

                for sb in range(SB):
                    s0 = sb * 512
                    hblk = hp_pool.tile([128, KC, 512], bf16)
                    for hh in range(2):
                        nc.sync.dma_start(
                            out=hblk[:, hh * 8:(hh + 1) * 8],
                            in_=hT[hh * 1024:(hh + 1) * 1024,
                                   s0:s0 + 512].rearrange(
                                "(c p) s -> p c s", p=128))
                    # ---- q projection (4 chunks of 128 rows) ----
                    for m in range(4):
                        ps = pps.tile([128, 512], f32, tag="proj")
                        for kc in range(KC):
                            nc.tensor.matmul(ps, wq_sb[:, kc, m * 128:(m + 1) * 128],
                                             hblk[:, kc, :],
                                             start=(kc == 0), stop=(kc == KC - 1))
                        sq_ps = sqps.tile([2, 512], f32, tag="sqs")
                        rb_ps = bcps.tile([128, 512], f32, tag="rbps")
                        r = (m // 2) * 64
                        cb = 2 * (m % 2)
                        rope_norm(ps, csA_q[:, s0:s0 + 512], csB_q[:, s0:s0 + 512],
                                  sq_ps, rb_ps,
                                  qT_sb[r:r + 64, cb, s0:s0 + 512],
                                  qT_sb[r:r + 64, cb + 1, s0:s0 + 512])
                    # ---- k projection (1 chunk) ----
                    ps = pps.tile([128, 512], f32, tag="proj")
                    for kc in range(KC):
                        nc.tensor.matmul(ps, wk_sb[:, kc, :], hblk[:, kc, :],
                                         start=(kc == 0), stop=(kc == KC - 1))
                    sq_ps = sqps.tile([2, 512], f32, tag="sqs")
                    rb_ps = bcps.tile([128, 512], f32, tag="rbps")
                    rope_norm(ps, csA_k[:, s0:s0 + 512], csB_k[:, s0:s0 + 512],
                              sq_ps, rb_ps,
                              kT_sb[0:64, s0:s0 + 512],
                              kT_sb[64:128, s0:s0 + 512])
                    # ---- v projection + transpose to natural layout ----
                    ps = pps.tile([128, 512], f32, tag="proj")
                    for kc in range(KC):
                        nc.tensor.matmul(ps, wv_sb[:, kc, :], hblk[:, kc, :],
                                         start=(kc == 0), stop=(kc == KC - 1))
                    vt = wk.tile([128, 512], bf16, tag="vt")
                    nc.scalar.copy(out=vt, in_=ps)
                    for ss in range(4):
                        tp = trps.tile([128, 128], bf16, tag="tp")
                        nc.tensor.transpose(tp, vt[:, ss * 128:(ss + 1) * 128],
                                            ident)
                        chunk = sb * 4 + ss
                        nc.vector.tensor_copy(out=v_sb[:, chunk, 0, 0:HD],
                                              in_=tp[:, 0:64])
                        nc.vector.tensor_copy(out=v_sb[:, chunk, 1, 0:HD],
                                              in_=tp[:, 64:128])

            # ========== phase 2: attention + gate/o_proj filler ==========
            from collections import deque
            filler = deque()

            with tc.tile_pool(name="wg2", bufs=1) as wgp, \
                 tc.tile_pool(name="hblk2", bufs=2) as hp2, \
                 tc.tile_pool(name="probs", bufs=2) as prp, \
                 tc.tile_pool(name="att_sm", bufs=2) as asm, \
                 tc.tile_pool(name="ostg", bufs=3) as ostg, \
                 tc.tile_pool(name="scps", bufs=2, space=PSUM) as scps, \
                 tc.tile_pool(name="avps", bufs=1, space=PSUM) as avps, \
                 tc.tile_pool(name="filps", bufs=2, space=PSUM) as filps:

                wg_sb = wgp.tile([128, KC, QD], bf16)
                nc.sync.dma_start(out=wg_sb,
                                  in_=wgT.rearrange("(c p) m -> p c m", p=128))

                # gate filler closures: 4 units per m-chunk (4 MMs each)
                def gate_dma(sb):
                    def run(state):
                        hblk = hp2.tile([128, KC, 512], bf16, tag="h2", name="hblk2")
                        nc.sync.dma_start(
                            out=hblk,
                            in_=hT[:, sb * 512:(sb + 1) * 512].rearrange(
                                "(c p) s -> p c s", p=128))
                        state[("hblk", sb)] = hblk
                    return run

                def gate_mms(sb, m, part):
                    def run(state):
                        if part == 0:
                            state["gps"] = filps.tile([128, 512], f32, tag="fil", name="gps")
                        ps = state["gps"]
                        hblk = state[("hblk", sb)]
                        for kc in range(part * 4, part * 4 + 4):
                            nc.tensor.matmul(ps, wg_sb[:, kc, m * 128:(m + 1) * 128],
                                             hblk[:, kc, :],
                                             start=(kc == 0), stop=(kc == KC - 1))
                        if part == 3:
                            s0 = sb * 512
                            graw = asm.tile([128, 512], bf16, tag="graw", name="graw")
                            nc.scalar.activation(out=graw, in_=ps, func=Tanh,
                                                 scale=0.5)
                            nc.vector.tensor_scalar(
                                out=g_sb[:, m, s0:s0 + 512], in0=graw,
                                scalar1=0.5, scalar2=0.5,
                                op0=mybir.AluOpType.mult,
                                op1=mybir.AluOpType.add)
                    return run

                # prefetch one hblk ahead (hblk2 pool has bufs=2)
                filler.append(gate_dma(0))
                for sb in range(SB):
                    if sb + 1 < SB:
                        filler.append(gate_dma(sb + 1))
                    for m in range(4):
                        for part in range(4):
                            filler.append(gate_mms(sb, m, part))

                def oproj_unit(m, nb):
                    def run(state):
                        po = filps.tile([128, 512], f32, tag="fil", name="po")
                        for oc in range(4):
                            nc.tensor.matmul(po, wo_sb[:, oc, m, :],
                                             og_sb[:, oc, nb * 512:(nb + 1) * 512],
                                             start=(oc == 0), stop=(oc == 3))
                        stg = ostg.tile([128, 512], bf16, tag="stg", name="stg")
                        nc.vector.tensor_copy(out=stg, in_=po)
                        nc.sync.dma_start(
                            out=outT[m * 128:(m + 1) * 128,
                                     nb * 512:(nb + 1) * 512],
                            in_=stg)
                    return run

                fstate = {}

                def pop_filler(n):
                    for _ in range(n):
                        if filler:
                            filler.popleft()(fstate)

                # warmup: throwaway score-pair round — the first matmul at
                # tile_position (64,0) after the K=128 projection stream has
                # been observed to produce garbage; absorb it here
                wsc = scps.tile([128, 2, 512], f32, tag="sc", name="wsc")
                for hi, kv in ((0, 0), (1, 1)):
                    nc.tensor.matmul(
                        wsc[:, hi, :],
                        kT_sb[kv * 64:kv * 64 + 64, 0:128],
                        qT_sb[kv * 64:kv * 64 + 64, 0, 0:512],
                        start=True, stop=True)
                wprobs = prp.tile([128, 2, 512], bf16, tag="probs",
                                  name="wprobs")
                nc.vector.tensor_copy(out=wprobs, in_=wsc)
                if debug_dump:
                    nc.sync.dma_start(out=dbg_wsc, in_=wprobs)

                # emit all sb=0 gate units up front: the first attention
                # tail reads g_sb chunks that must already have writers in
                # the instruction DAG, else no dependency is created and
                # the gating multiply reads uninitialized SBUF
                pop_filler(18)

                # attention rounds: q-block-major, head pairs (hp, hp+4)
                for gb in range(SB):
                    q0 = gb * 512
                    nkc = 4 * gb + 4
                    for hpi in range(4):
                        av = avps.tile([65, 2, 512], f32, tag="av")
                        probs_tiles = {}
                        prev = None
                        for kc in range(nkc):
                            sQ = max(0, 128 * kc - q0)
                            sc = scps.tile([128, 2, 512], f32, tag="sc")
                            probs = prp.tile([128, 2, 512], bf16, tag="probs")
                            probs_tiles[kc] = (probs, sQ)
                            # paired score matmuls (disjoint PE row groups)
                            for hi, kv in ((0, 0), (1, 1)):
                                nc.tensor.matmul(
                                    sc[:, hi, sQ:512],
                                    kT_sb[kv * 64:kv * 64 + 64,
                                          kc * 128:(kc + 1) * 128],
                                    qT_sb[kv * 64:kv * 64 + 64, hpi,
                                          q0 + sQ:q0 + 512],
                                    start=True, stop=True)
                            nc.scalar.activation(out=probs[:, :, sQ:512],
                                                 in_=sc[:, :, sQ:512],
                                                 func=Exp, scale=SCALE)
                            if kc >= 4 * gb:  # diagonal chunk: mask 128 cols
                                for hi in range(2):
                                    nc.vector.tensor_tensor(
                                        out=probs[:, hi, sQ:sQ + 128],
                                        in0=probs[:, hi, sQ:sQ + 128],
                                        in1=tri, op=mybir.AluOpType.min)
                            if prev is not None:
                                pkc = prev
                                pprobs, psQ = probs_tiles.pop(pkc)
                                for hi, kv in ((0, 0), (1, 1)):
                                    nc.tensor.matmul(
                                        av[:, hi, psQ:512],
                                        v_sb[:, pkc, kv, :],
                                        pprobs[:, hi, psQ:512],
                                        start=(pkc == 0), stop=False)
                            prev = kc
                            pop_filler(2)
                        pprobs, psQ = probs_tiles.pop(prev)
                        for hi, kv in ((0, 0), (1, 1)):
                            nc.tensor.matmul(av[:, hi, psQ:512],
                                             v_sb[:, prev, kv, :],
                                             pprobs[:, hi, psQ:512],
                                             start=(prev == 0), stop=True)
                        # tail: denominators, broadcast, gating
                        den = asm.tile([1, 2, 512], f32, tag="den")
                        nc.vector.tensor_copy(out=den, in_=av[64:65, :, :])
                        recf = asm.tile([1, 2, 512], f32, tag="recf")
                        nc.vector.reciprocal_approx_fast(out=recf, in_=den)
                        recb = asm.tile([1, 2, 512], bf16, tag="recb")
                        nc.vector.tensor_copy(out=recb, in_=recf)
                        avc = asm.tile([64, 2, 512], bf16, tag="avc")
                        nc.vector.tensor_copy(out=avc, in_=av[0:64, :, :])
                        rbv = asm.tile([64, 2, 512], bf16, tag="rbv")
                        nc.gpsimd.partition_broadcast(rbv, recb)
                        hp_ = (hpi % 2) * 64
                        for hi in range(2):
                            hc = hpi // 2 + 2 * hi
                            dst = og_sb[hp_:hp_ + 64, hc, q0:q0 + 512]
                            # both SBUF inputs at base 0; only the output
                            # partition base differs (allowed)
                            nc.vector.tensor_mul(dst, avc[:, hi, :],
                                                 rbv[:, hi, :])
                            nc.vector.tensor_mul(
                                dst, dst, g_sb[hp_:hp_ + 64, hc, q0:q0 + 512])
                        pop_filler(1)
                    # og for q-block gb complete -> o_proj for nb=gb
                    for m in range(KC):
                        filler.append(oproj_unit(m, gb))

                while filler:
                    filler.popleft()(fstate)

            if debug_dump:
                nc.sync.dma_start(out=dbg_q, in_=qT_sb)
                nc.sync.dma_start(out=dbg_k, in_=kT_sb)
                nc.sync.dma_start(out=dbg_v, in_=v_sb)
                nc.sync.dma_start(out=dbg_g, in_=g_sb)
                nc.sync.dma_start(out=dbg_og, in_=og_sb)

    nc.compile()
    return nc


def _host_prep(hidden_states, cos, sin, Wq, Wk, Wv, Wg, Wo, q_norm_w, k_norm_w):
    """Build per-core input maps."""
    def cs_tables(cos_b, sin_b, w):
        A = np.empty((128, S), np.float32)
        Bt = np.empty((128, S), np.float32)
        cosT = cos_b.T  # [32, S]
        sinT = sin_b.T
        for blk in (0, 64):
            A[blk + 0:blk + 32] = cosT * w[0:32, None]
            A[blk + 32:blk + 64] = w[32:64, None]
            Bt[blk + 0:blk + 16] = -sinT[0:16] * w[16:32, None]
            Bt[blk + 16:blk + 32] = sinT[16:32] * w[0:16, None]
            Bt[blk + 32:blk + 64] = 0.0
        return A.astype(BF16), Bt.astype(BF16)

    in_maps = []
    for c in range(NCORES):
        b, g = c // 4, c % 4
        qs = slice(g * QD, (g + 1) * QD)
        ks = slice(g * KD, (g + 1) * KD)
        csA_q, csB_q = cs_tables(cos[b], sin[b], np.asarray(q_norm_w))
        csA_k, csB_k = cs_tables(cos[b], sin[b], np.asarray(k_norm_w))
        in_maps.append({
            "hT": np.ascontiguousarray(hidden_states[b].T).astype(BF16),
            "wqT": np.ascontiguousarray(Wq[qs].T).astype(BF16),
            "wkT": np.ascontiguousarray(Wk[ks].T).astype(BF16),
            "wvT": np.ascontiguousarray(Wv[ks].T).astype(BF16),
            "wgT": np.ascontiguousarray(Wg[qs].T).astype(BF16),
            "woT": np.ascontiguousarray(Wo[:, qs].T).astype(BF16),
            "csAq": csA_q, "csBq": csB_q, "csAk": csA_k, "csBk": csB_k,
        })
    return in_maps


def kernel(hidden_states, cos, sin, Wq, Wk, Wv, Wg, Wo, q_norm_w, k_norm_w):
    from concourse import bass_utils

    dbg = bool(int(os.environ.get("KERNEL_DEBUG", "0")))
    if "nc" not in _CACHE:
        _CACHE["nc"] = _build_bass(debug_dump=dbg)
    nc = _CACHE["nc"]

    in_maps = _host_prep(hidden_states, cos, sin, Wq, Wk, Wv, Wg, Wo,
                         q_norm_w, k_norm_w)

    trace = bool(int(os.environ.get("KERNEL_TRACE", "0")))
    kwargs = {}
    if trace:
        try:
            import antenv.axon_hooks  # noqa: F401
        except ImportError:
            import types
            sys.path.insert(0, "/root/.axon_site")
            from trn_agent_boot.trn_boot import _ntff_profile_via_ctypes
            hook = _ntff_profile_via_ctypes("/opt/axon/libaxon_pjrt.so")
            mod = types.ModuleType("antenv.axon_hooks")
            mod.get_axon_ntff_profile_hook = lambda: hook
            sys.modules["antenv.axon_hooks"] = mod
        tmpdir = os.environ.get("KERNEL_TRACE_DIR") or None
        kwargs = dict(trace=True, tmpdir=tmpdir)
    res = bass_utils.run_bass_kernel_spmd(nc, in_maps,
                                          core_ids=list(range(NCORES)),
                                          **kwargs)
    if trace and res.exec_time_ns is not None:
        print(f"HW exec time: {res.exec_time_ns} ns")
        _CACHE["exec_time_ns"] = res.exec_time_ns

    if dbg:
        _CACHE["dumps"] = res.results

    out = np.zeros((B, S, HID), np.float32)
    for c in range(NCORES):
        b = c // 4
        out[b] += res.results[c]["outT"].astype(np.float32).T
    return out


if __name__ == "__main__":
    rng = np.random.default_rng(0)
    hs = rng.standard_normal((B, S, HID), dtype=np.float32)
    cos = rng.random((B, S, ROPE), dtype=np.float32)
    sin = rng.random((B, S, ROPE), dtype=np.float32)
    out = kernel(hidden_states=hs, cos=cos, sin=sin,
                 Wq=rng.standard_normal((NH * HD, HID), dtype=np.float32) * 0.02,
                 Wk=rng.standard_normal((NKV * HD, HID), dtype=np.float32) * 0.02,
                 Wv=rng.standard_normal((NKV * HD, HID), dtype=np.float32) * 0.02,
                 Wg=rng.standard_normal((NH * HD, HID), dtype=np.float32) * 0.02,
                 Wo=rng.standard_normal((HID, NH * HD), dtype=np.float32) * 0.02,
                 q_norm_w=np.ones(HD, np.float32),
                 k_norm_w=np.ones(HD, np.float32))
    print(out.shape, out.dtype)


# revision 4
# speedup vs baseline: 1.0095x; 1.0095x over previous
"""GQA attention Trainium2 kernel v2 (8 NeuronCores, SPMD, no collectives).

Sharding: 2-way data parallel (batch) x 4-way tensor parallel (heads).
Core c handles batch b=c//4 and head-group g=c%4 (8 q heads, 2 kv heads).

v2 changes vs baseline:
- reciprocal_approx_fast instead of iterative reciprocal (was 171us busy).
- Attention restructured q-block-major with head pairs (kv0+kv1) so score
  matmuls pack into disjoint PE row groups and run concurrently.
- exp calls merged across the head pair; diagonal blocks trimmed so
  fully-masked columns are neither exp'd nor streamed through PV.
- Causal mask via precomputed [128,128] triangular mask + vector multiply
  (replaces per-block gpsimd affine_select).
- Gate projection and 3/4 of o_proj interleaved into the attention phase
  as tensor-engine filler (attention alone is scalar-bound).
- sigmoid(x) computed as 0.5*tanh(x/2)+0.5 to stay in the exp activation
  table set (avoids ACT_TABLE_LOAD thrash).
- Phase-1 elementwise work in bf16 (2x DVE mode); bf16 output.
"""

import os
import sys
import numpy as np

for _p in ("/opt/trn_rl_repo", "/root/.axon_site/_ro/trn_rl_repo"):
    if os.path.isdir(_p) and _p not in sys.path:
        sys.path.insert(0, _p)

import ml_dtypes

B, S, HID = 2, 2048, 2048
NH, NKV, HD = 32, 8, 64
ROPE = 32
SCALE = HD ** -0.5
NCORES = 8
QH = NH // 4      # 8 q heads per core
KVH = NKV // 4    # 2 kv heads per core
QD = QH * HD      # 512 per-core q dim
KD = KVH * HD     # 128 per-core kv dim
KC = HID // 128   # 16 contraction chunks
SB = S // 512     # 4 sequence blocks of 512
BF16 = ml_dtypes.bfloat16

_CACHE = {}


def _build_bass(debug_dump=False):
    import concourse.bass as bass
    from concourse import bacc, mybir, tile
    from concourse.masks import make_identity

    f32 = mybir.dt.float32
    bf16 = mybir.dt.bfloat16

    nc = bacc.Bacc("TRN2", target_bir_lowering=False, debug=False,
                   enable_asserts=False, num_devices=NCORES)

    hT = nc.dram_tensor("hT", [HID, S], bf16, kind="ExternalInput").ap()
    wqT = nc.dram_tensor("wqT", [HID, QD], bf16, kind="ExternalInput").ap()
    wkT = nc.dram_tensor("wkT", [HID, KD], bf16, kind="ExternalInput").ap()
    wvT = nc.dram_tensor("wvT", [HID, KD], bf16, kind="ExternalInput").ap()
    wgT = nc.dram_tensor("wgT", [HID, QD], bf16, kind="ExternalInput").ap()
    woT = nc.dram_tensor("woT", [QD, HID], bf16, kind="ExternalInput").ap()
    csAq = nc.dram_tensor("csAq", [128, S], bf16, kind="ExternalInput").ap()
    csBq = nc.dram_tensor("csBq", [128, S], bf16, kind="ExternalInput").ap()
    csAk = nc.dram_tensor("csAk", [128, S], bf16, kind="ExternalInput").ap()
    csBk = nc.dram_tensor("csBk", [128, S], bf16, kind="ExternalInput").ap()
    outT = nc.dram_tensor("outT", [HID, S], bf16, kind="ExternalOutput").ap()
    if debug_dump:
        dbg_q = nc.dram_tensor("dbg_q", [128, 4, S], bf16, kind="ExternalOutput").ap()
        dbg_k = nc.dram_tensor("dbg_k", [128, S], bf16, kind="ExternalOutput").ap()
        dbg_v = nc.dram_tensor("dbg_v", [128, KC, KVH, HD + 1], bf16,
                               kind="ExternalOutput").ap()
        dbg_g = nc.dram_tensor("dbg_g", [128, 4, S], bf16, kind="ExternalOutput").ap()
        dbg_og = nc.dram_tensor("dbg_og", [128, 4, S], bf16,
                                kind="ExternalOutput").ap()

    Exp = mybir.ActivationFunctionType.Exp
    Tanh = mybir.ActivationFunctionType.Tanh
    Square = mybir.ActivationFunctionType.Square
    Sqrt = mybir.ActivationFunctionType.Sqrt
    PSUM = bass.MemorySpace.PSUM

    with tile.TileContext(nc) as tc:
        with tc.tile_pool(name="persist", bufs=1) as pp:
            # head h: partition rows (h//4)*64, free chunk h%4 (q side);
            # og/gate: head h at rows (h%2)*64, chunk h//2
            qT_sb = pp.tile([128, 4, S], bf16)
            kT_sb = pp.tile([128, S], bf16)
            g_sb = pp.tile([128, 4, S], bf16)
            v_sb = pp.tile([128, KC, KVH, HD + 1], bf16)
            wo_sb = pp.tile([128, 4, KC, 128], bf16)
            og_sb = pp.tile([128, 4, S], bf16)
            tri = pp.tile([128, 128], bf16)          # tri[r,c]=1 if c>=r

            nc.vector.memset(v_sb[:, :, :, HD:HD + 1], 1.0)
            # tri: +inf on keep (c>=r), 0 on mask; applied via tensor-min so
            # masked inf/garbage becomes 0 instead of NaN (inf*0)
            nc.vector.memset(tri, float("inf"))
            nc.gpsimd.affine_select(out=tri, in_=tri,
                                    compare_op=mybir.AluOpType.is_ge,
                                    fill=0.0, base=0, channel_multiplier=-1,
                                    pattern=[[1, 128]])

            # ================= phase 1: q/k/v projections =================
            with tc.tile_pool(name="consts", bufs=1) as cp, \
                 tc.tile_pool(name="wts", bufs=1) as wp, \
                 tc.tile_pool(name="hblk", bufs=2) as hp_pool, \
                 tc.tile_pool(name="work", bufs=3) as wk, \
                 tc.tile_pool(name="smallw", bufs=3) as smp, \
                 tc.tile_pool(name="rbpool", bufs=2) as rbp, \
                 tc.tile_pool(name="rwork", bufs=3) as rwk, \
                 tc.tile_pool(name="pps", bufs=2, space=PSUM) as pps, \
                 tc.tile_pool(name="sqps", bufs=2, space=PSUM) as sqps, \
                 tc.tile_pool(name="bcps", bufs=2, space=PSUM) as bcps, \
                 tc.tile_pool(name="trps", bufs=2, space=PSUM) as trps:

                csA_q = cp.tile([128, S], bf16)
                csB_q = cp.tile([128, S], bf16)
                csA_k = cp.tile([128, S], bf16)
                csB_k = cp.tile([128, S], bf16)
                nc.sync.dma_start(out=csA_q, in_=csAq)
                nc.sync.dma_start(out=csB_q, in_=csBq)
                nc.sync.dma_start(out=csA_k, in_=csAk)
                nc.sync.dma_start(out=csB_k, in_=csBk)
                ident = cp.tile([128, 128], bf16)
                make_identity(nc, ident)
                ones2 = cp.tile([128, 2], bf16)
                nc.vector.memset(ones2, 0.0)
                nc.vector.memset(ones2[0:64, 0:1], 1.0)
                nc.vector.memset(ones2[64:128, 1:2], 1.0)
                # bsel: matmul-broadcast selector; out row m <- rstd row (m>=64)
                bsel = cp.tile([2, 128], bf16)
                nc.vector.memset(bsel, 0.0)
                nc.vector.memset(bsel[0:1, 0:64], 1.0)
                # partition base 1 is not DVE-addressable; fill row 1 via DMA
                nc.gpsimd.dma_start(out=bsel[1:2, 64:128], in_=bsel[0:1, 0:64])

                wq_sb = wp.tile([128, KC, QD], bf16)
                wk_sb = wp.tile([128, KC, KD], bf16)
                wv_sb = wp.tile([128, KC, KD], bf16)
                nc.sync.dma_start(out=wq_sb,
                                  in_=wqT.rearrange("(c p) m -> p c m", p=128))
                nc.sync.dma_start(out=wk_sb,
                                  in_=wkT.rearrange("(c p) m -> p c m", p=128))
                nc.sync.dma_start(out=wv_sb,
                                  in_=wvT.rearrange("(c p) m -> p c m", p=128))
                nc.sync.dma_start(out=wo_sb,
                                  in_=woT.rearrange("(c p) (mb mm) -> p c mb mm",
                                                    p=128, mm=128))

                def rope_norm(ps, csA, csB, sq_ps, rb_ps, dst0, dst1):
                    """ps: psum [128,512] raw proj (2 heads of 64 dims).
                    Writes normalized+roped bf16 rows to dst0 (rows 0:64)
                    and dst1 (rows 64:128)."""
                    # sum of squares per head-half via ones2 matmul on bf16
                    # square tile
                    sq_t = wk.tile([128, 512], bf16, tag="sq")
                    nc.scalar.activation(out=sq_t, in_=ps, func=Square)
                    qsb = rwk.tile([128, 512], bf16, tag="qsb")
                    nc.scalar.copy(out=qsb, in_=ps)
                    nc.tensor.matmul(sq_ps, ones2, sq_t, start=True, stop=True)
                    # rstd = sqrt(HD / sum_sq)
                    inv_ms = smp.tile([2, 512], f32, tag="invms")
                    nc.vector.reciprocal_approx_fast(out=inv_ms, in_=sq_ps)
                    rstdb = smp.tile([2, 512], bf16, tag="rstdb")
                    nc.scalar.activation(out=rstdb, in_=inv_ms, func=Sqrt,
                                         scale=float(HD))
                    # broadcast rstd rows to 64-row halves via tiny matmul
                    nc.tensor.matmul(rb_ps, bsel, rstdb, start=True, stop=True)
                    rb = rbp.tile([128, 512], bf16, tag="rb")
                    nc.scalar.copy(out=rb, in_=rb_ps)
                    # rope: qa = qsb*csA + rot(qsb)*csB  (bf16, 2x DVE)
                    qa = rwk.tile([128, 512], bf16, tag="qa")
                    nc.vector.tensor_mul(qa, qsb, csA)
                    rot = rwk.tile([128, 512], bf16, tag="rot")
                    nc.vector.memset(rot[32:64, :], 0.0)
                    nc.vector.memset(rot[96:128, :], 0.0)
                    for hh in (0, 64):
                        nc.gpsimd.dma_start(out=rot[hh + 0:hh + 16],
                                            in_=qsb[hh + 16:hh + 32])
                        nc.gpsimd.dma_start(out=rot[hh + 16:hh + 32],
                                            in_=qsb[hh + 0:hh + 16])
                    nc.vector.tensor_mul(rot, rot, csB)
                    nc.vector.tensor_add(qa, qa, rot)
                    nc.vector.tensor_mul(dst0, qa[0:64, :], rb[0:64, :])
                    nc.vector.tensor_mul(dst1, qa[64:128, :], rb[64:128, :])

                for sb in range(SB):
                    s0 = sb * 512
                    hblk = hp_pool.tile([128, KC, 512], bf16)
                    for hh in range(2):
                        nc.sync.dma_start(
                            out=hblk[:, hh * 8:(hh + 1) * 8],
                            in_=hT[hh * 1024:(hh + 1) * 1024,
                                   s0:s0 + 512].rearrange(
                                "(c p) s -> p c s", p=128))
                    # ---- q projection (4 chunks of 128 rows) ----
                    for m in range(4):
                        ps = pps.tile([128, 512], f32, tag="proj")
                        for kc in range(KC):
                            nc.tensor.matmul(ps, wq_sb[:, kc, m * 128:(m + 1) * 128],
                                             hblk[:, kc, :],
                                             start=(kc == 0), stop=(kc == KC - 1))
                        sq_ps = sqps.tile([2, 512], f32, tag="sqs")
                        rb_ps = bcps.tile([128, 512], f32, tag="rbps")
                        r = (m // 2) * 64
                        cb = 2 * (m % 2)
                        rope_norm(ps, csA_q[:, s0:s0 + 512], csB_q[:, s0:s0 + 512],
                                  sq_ps, rb_ps,
                                  qT_sb[r:r + 64, cb, s0:s0 + 512],
                                  qT_sb[r:r + 64, cb + 1, s0:s0 + 512])
                    # ---- k projection (1 chunk) ----
                    ps = pps.tile([128, 512], f32, tag="proj")
                    for kc in range(KC):
                        nc.tensor.matmul(ps, wk_sb[:, kc, :], hblk[:, kc, :],
                                         start=(kc == 0), stop=(kc == KC - 1))
                    sq_ps = sqps.tile([2, 512], f32, tag="sqs")
                    rb_ps = bcps.tile([128, 512], f32, tag="rbps")
                    rope_norm(ps, csA_k[:, s0:s0 + 512], csB_k[:, s0:s0 + 512],
                              sq_ps, rb_ps,
                              kT_sb[0:64, s0:s0 + 512],
                              kT_sb[64:128, s0:s0 + 512])
                    # ---- v projection + transpose to natural layout ----
                    ps = pps.tile([128, 512], f32, tag="proj")
                    for kc in range(KC):
                        nc.tensor.matmul(ps, wv_sb[:, kc, :], hblk[:, kc, :],
                                         start=(kc == 0), stop=(kc == KC - 1))
                    vt = wk.tile([128, 512], bf16, tag="vt")
                    nc.scalar.copy(out=vt, in_=ps)
                    for ss in range(4):
                        tp = trps.tile([128, 128], bf16, tag="tp")
                        nc.tensor.transpose(tp, vt[:, ss * 128:(ss + 1) * 128],
                                            ident)
                        chunk = sb * 4 + ss
                        nc.vector.tensor_copy(out=v_sb[:, chunk, 0, 0:HD],
                                              in_=tp[:, 0:64])
                        nc.vector.tensor_copy(out=v_sb[:, chunk, 1, 0:HD],
                                              in_=tp[:, 64:128])

            # ========== phase 2: attention + gate/o_proj filler ==========
            from collections import deque
            filler = deque()

            with tc.tile_pool(name="wg2", bufs=1) as wgp, \
                 tc.tile_pool(name="hblk2", bufs=2) as hp2, \
                 tc.tile_pool(name="probs", bufs=2) as prp, \
                 tc.tile_pool(name="att_sm", bufs=2) as asm, \
                 tc.tile_pool(name="ostg", bufs=3) as ostg, \
                 tc.tile_pool(name="scps", bufs=2, space=PSUM) as scps, \
                 tc.tile_pool(name="avps", bufs=1, space=PSUM) as avps, \
                 tc.tile_pool(name="filps", bufs=2, space=PSUM) as filps:

                wg_sb = wgp.tile([128, KC, QD], bf16)
                nc.sync.dma_start(out=wg_sb,
                                  in_=wgT.rearrange("(c p) m -> p c m", p=128))

                # gate filler closures: 4 units per m-chunk (4 MMs each)
                def gate_dma(sb):
                    def run(state):
                        hblk = hp2.tile([128, KC, 512], bf16, tag="h2", name="hblk2")
                        nc.sync.dma_start(
                            out=hblk,
                            in_=hT[:, sb * 512:(sb + 1) * 512].rearrange(
                                "(c p) s -> p c s", p=128))
                        state[("hblk", sb)] = hblk
                    return run

                def gate_mms(sb, m, part):
                    def run(state):
                        if part == 0:
                            state["gps"] = filps.tile([128, 512], f32, tag="fil", name="gps")
                        ps = state["gps"]
                        hblk = state[("hblk", sb)]
                        for kc in range(part * 4, part * 4 + 4):
                            nc.tensor.matmul(ps, wg_sb[:, kc, m * 128:(m + 1) * 128],
                                             hblk[:, kc, :],
                                             start=(kc == 0), stop=(kc == KC - 1))
                        if part == 3:
                            s0 = sb * 512
                            graw = asm.tile([128, 512], bf16, tag="graw", name="graw")
                            nc.scalar.activation(out=graw, in_=ps, func=Tanh,
                                                 scale=0.5)
                            nc.vector.tensor_scalar(
                                out=g_sb[:, m, s0:s0 + 512], in0=graw,
                                scalar1=0.5, scalar2=0.5,
                                op0=mybir.AluOpType.mult,
                                op1=mybir.AluOpType.add)
                    return run

                # prefetch one hblk ahead (hblk2 pool has bufs=2)
                filler.append(gate_dma(0))
                for sb in range(SB):
                    if sb + 1 < SB:
                        filler.append(gate_dma(sb + 1))
                    for m in range(4):
                        for part in range(4):
                            filler.append(gate_mms(sb, m, part))

                def oproj_unit(m, nb):
                    def run(state):
                        po = filps.tile([128, 512], f32, tag="fil", name="po")
                        for oc in range(4):
                            nc.tensor.matmul(po, wo_sb[:, oc, m, :],
                                             og_sb[:, oc, nb * 512:(nb + 1) * 512],
                                             start=(oc == 0), stop=(oc == 3))
                        stg = ostg.tile([128, 512], bf16, tag="stg", name="stg")
                        nc.vector.tensor_copy(out=stg, in_=po)
                        nc.sync.dma_start(
                            out=outT[m * 128:(m + 1) * 128,
                                     nb * 512:(nb + 1) * 512],
                            in_=stg)
                    return run

                fstate = {"popped": 0}

                def pop_filler(n):
                    for _ in range(n):
                        if filler:
                            filler.popleft()(fstate)
                            fstate["popped"] += 1

                # filler index after which every gate unit for seq blocks
                # <= gb is emitted (gate units precede o_proj units in the
                # deque, so a plain pop count suffices)
                GATE_END = {0: 18, 1: 35, 2: 52, 3: 68}

                # attention rounds: q-block-major, head pairs (hp, hp+4)
                for gb in range(SB):
                    q0 = gb * 512
                    nkc = 4 * gb + 4
                    for hpi in range(4):
                        av = avps.tile([65, 2, 512], f32, tag="av")
                        probs_tiles = {}
                        prev = None
                        for kc in range(nkc):
                            sQ = max(0, 128 * kc - q0)
                            sc = scps.tile([128, 2, 512], f32, tag="sc")
                            probs = prp.tile([128, 2, 512], bf16, tag="probs")
                            probs_tiles[kc] = (probs, sQ)
                            # paired score matmuls (disjoint PE row groups)
                            for hi, kv in ((0, 0), (1, 1)):
                                nc.tensor.matmul(
                                    sc[:, hi, sQ:512],
                                    kT_sb[kv * 64:kv * 64 + 64,
                                          kc * 128:(kc + 1) * 128],
                                    qT_sb[kv * 64:kv * 64 + 64, hpi,
                                          q0 + sQ:q0 + 512],
                                    start=True, stop=True)
                            nc.scalar.activation(out=probs[:, :, sQ:512],
                                                 in_=sc[:, :, sQ:512],
                                                 func=Exp, scale=SCALE)
                            if kc >= 4 * gb:  # diagonal chunk: mask 128 cols
                                for hi in range(2):
                                    nc.vector.tensor_tensor(
                                        out=probs[:, hi, sQ:sQ + 128],
                                        in0=probs[:, hi, sQ:sQ + 128],
                                        in1=tri, op=mybir.AluOpType.min)
                            if prev is not None:
                                pkc = prev
                                pprobs, psQ = probs_tiles.pop(pkc)
                                for hi, kv in ((0, 0), (1, 1)):
                                    nc.tensor.matmul(
                                        av[:, hi, psQ:512],
                                        v_sb[:, pkc, kv, :],
                                        pprobs[:, hi, psQ:512],
                                        start=(pkc == 0), stop=False)
                            prev = kc
                            pop_filler(2)
                        pprobs, psQ = probs_tiles.pop(prev)
                        for hi, kv in ((0, 0), (1, 1)):
                            nc.tensor.matmul(av[:, hi, psQ:512],
                                             v_sb[:, prev, kv, :],
                                             pprobs[:, hi, psQ:512],
                                             start=(prev == 0), stop=True)
                        # guard: every gate unit this tail reads must be
                        # emitted already, else no dependency edge exists
                        # and the gating multiply reads stale SBUF
                        pop_filler(max(0, GATE_END[gb] - fstate["popped"]))
                        # tail: denominators, broadcast, gating
                        den = asm.tile([1, 2, 512], f32, tag="den")
                        nc.vector.tensor_copy(out=den, in_=av[64:65, :, :])
                        recf = asm.tile([1, 2, 512], f32, tag="recf")
                        nc.vector.reciprocal_approx_fast(out=recf, in_=den)
                        recb = asm.tile([1, 2, 512], bf16, tag="recb")
                        nc.vector.tensor_copy(out=recb, in_=recf)
                        avc = asm.tile([64, 2, 512], bf16, tag="avc")
                        nc.vector.tensor_copy(out=avc, in_=av[0:64, :, :])
                        rbv = asm.tile([64, 2, 512], bf16, tag="rbv")
                        nc.gpsimd.partition_broadcast(rbv, recb)
                        hp_ = (hpi % 2) * 64
                        for hi in range(2):
                            hc = hpi // 2 + 2 * hi
                            dst = og_sb[hp_:hp_ + 64, hc, q0:q0 + 512]
                            # both SBUF inputs at base 0; only the output
                            # partition base differs (allowed)
                            nc.vector.tensor_mul(dst, avc[:, hi, :],
                                                 rbv[:, hi, :])
                            nc.vector.tensor_mul(
                                dst, dst, g_sb[hp_:hp_ + 64, hc, q0:q0 + 512])
                        pop_filler(1)
                    # og for q-block gb complete -> o_proj for nb=gb
                    for m in range(KC):
                        filler.append(oproj_unit(m, gb))

                while filler:
                    filler.popleft()(fstate)

            if debug_dump:
                nc.sync.dma_start(out=dbg_q, in_=qT_sb)
                nc.sync.dma_start(out=dbg_k, in_=kT_sb)
                nc.sync.dma_start(out=dbg_v, in_=v_sb)
                nc.sync.dma_start(out=dbg_g, in_=g_sb)
                nc.sync.dma_start(out=dbg_og, in_=og_sb)

    nc.compile()
    return nc


def _host_prep(hidden_states, cos, sin, Wq, Wk, Wv, Wg, Wo, q_norm_w, k_norm_w):
    """Build per-core input maps."""
    def cs_tables(cos_b, sin_b, w):
        A = np.empty((128, S), np.float32)
        Bt = np.empty((128, S), np.float32)
        cosT = cos_b.T  # [32, S]
        sinT = sin_b.T
        for blk in (0, 64):
            A[blk + 0:blk + 32] = cosT * w[0:32, None]
            A[blk + 32:blk + 64] = w[32:64, None]
            Bt[blk + 0:blk + 16] = -sinT[0:16] * w[16:32, None]
            Bt[blk + 16:blk + 32] = sinT[16:32] * w[0:16, None]
            Bt[blk + 32:blk + 64] = 0.0
        return A.astype(BF16), Bt.astype(BF16)

    in_maps = []
    for c in range(NCORES):
        b, g = c // 4, c % 4
        qs = slice(g * QD, (g + 1) * QD)
        ks = slice(g * KD, (g + 1) * KD)
        csA_q, csB_q = cs_tables(cos[b], sin[b], np.asarray(q_norm_w))
        csA_k, csB_k = cs_tables(cos[b], sin[b], np.asarray(k_norm_w))
        in_maps.append({
            "hT": np.ascontiguousarray(hidden_states[b].T).astype(BF16),
            "wqT": np.ascontiguousarray(Wq[qs].T).astype(BF16),
            "wkT": np.ascontiguousarray(Wk[ks].T).astype(BF16),
            "wvT": np.ascontiguousarray(Wv[ks].T).astype(BF16),
            "wgT": np.ascontiguousarray(Wg[qs].T).astype(BF16),
            "woT": np.ascontiguousarray(Wo[:, qs].T).astype(BF16),
            "csAq": csA_q, "csBq": csB_q, "csAk": csA_k, "csBk": csB_k,
        })
    return in_maps


def kernel(hidden_states, cos, sin, Wq, Wk, Wv, Wg, Wo, q_norm_w, k_norm_w):
    from concourse import bass_utils

    dbg = bool(int(os.environ.get("KERNEL_DEBUG", "0")))
    if "nc" not in _CACHE:
        _CACHE["nc"] = _build_bass(debug_dump=dbg)
    nc = _CACHE["nc"]

    in_maps = _host_prep(hidden_states, cos, sin, Wq, Wk, Wv, Wg, Wo,
                         q_norm_w, k_norm_w)

    trace = bool(int(os.environ.get("KERNEL_TRACE", "0")))
    kwargs = {}
    if trace:
        try:
            import antenv.axon_hooks  # noqa: F401
        except ImportError:
            import types
            sys.path.insert(0, "/root/.axon_site")
            from trn_agent_boot.trn_boot import _ntff_profile_via_ctypes
            hook = _ntff_profile_via_ctypes("/opt/axon/libaxon_pjrt.so")
            mod = types.ModuleType("antenv.axon_hooks")
            mod.get_axon_ntff_profile_hook = lambda: hook
            sys.modules["antenv.axon_hooks"] = mod
        tmpdir = os.environ.get("KERNEL_TRACE_DIR") or None
        kwargs = dict(trace=True, tmpdir=tmpdir)
    res = bass_utils.run_bass_kernel_spmd(nc, in_maps,
                                          core_ids=list(range(NCORES)),
                                          **kwargs)
    if trace and res.exec_time_ns is not None:
        print(f"HW exec time: {res.exec_time_ns} ns")
        _CACHE["exec_time_ns"] = res.exec_time_ns

    if dbg:
        _CACHE["dumps"] = res.results

    out = np.zeros((B, S, HID), np.float32)
    for c in range(NCORES):
        b = c // 4
        out[b] += res.results[c]["outT"].astype(np.float32).T
    return out


if __name__ == "__main__":
    rng = np.random.default_rng(0)
    hs = rng.standard_normal((B, S, HID), dtype=np.float32)
    cos = rng.random((B, S, ROPE), dtype=np.float32)
    sin = rng.random((B, S, ROPE), dtype=np.float32)
    out = kernel(hidden_states=hs, cos=cos, sin=sin,
                 Wq=rng.standard_normal((NH * HD, HID), dtype=np.float32) * 0.02,
                 Wk=rng.standard_normal((NKV * HD, HID), dtype=np.float32) * 0.02,
                 Wv=rng.standard_normal((NKV * HD, HID), dtype=np.float32) * 0.02,
                 Wg=rng.standard_normal((NH * HD, HID), dtype=np.float32) * 0.02,
                 Wo=rng.standard_normal((HID, NH * HD), dtype=np.float32) * 0.02,
                 q_norm_w=np.ones(HD, np.float32),
                 k_norm_w=np.ones(HD, np.float32))
    print(out.shape, out.dtype)
